# revision 5
# baseline (speedup 1.0000x reference)
"""
Trainium2 Bass kernel for nn_ClusterLoss (vq_codebook):
    out = mean((X - decoding)^2) + ALPHA * soft_kmeans_loss(encoding, K=64)

Strategy (8 NeuronCores, data-parallel over the N=65536 sample axis):
  - Each core holds an 8192-row shard of X / decoding / encoding.
  - Decoder MSE: stream X/decoding shards, y = x - d on VectorE, fused
    Square+row-accumulate on ScalarE -> per-partition partial sums.
  - Soft k-means (10 iters, K=D=64): encoding shard stays SBUF-resident in two
    layouts (row chunks for the r.T@enc matmul; transposed-with-ones-row for
    the distance matmul).  Per iteration:
      m[n,k] = 2*enc[n]@C[k] - |C[k]|^2           (PE, homogeneous-coord trick)
      r = softmax(m) per sample                    (DVE reduce_max(negate) +
                                                    ScalarE fused exp/bias/accum)
      [r.T@enc | sum(r)] accumulated in PSUM       (PE, ones column trick)
      16.6KB AllReduce over 8 cores, C = num/(den+eps) on-device.
    Only iteration 10's loss is used by the reference (and its centroid update
    is dead), so iters 1..9 do stats-only and iter 10 does loss-only.
  - Host combines per-core [128,2] partial sums in float64.
"""

import sys

sys.path.insert(0, "/opt/trn_rl_repo")

import numpy as np

import concourse.bass as bass  # noqa: F401  (registers types)
import concourse.bacc as bacc
import concourse.tile as tile
from concourse import mybir
from concourse.bass_utils import run_bass_kernel_spmd
from concourse.masks import make_identity

ALPHA = 0.001
BETA = 1.0
N_ITERS = 10
EPS = 1e-8

NCORES = 8
N = 65536
D_DATA = 512
D_LAT = 64
K = 64
NLOC = N // NCORES          # 8192 samples per core
NCHUNK = NLOC // 128        # 64 chunks of 128 samples
NGRP = 8                    # chunks per PSUM group tile
NMSE = 16                   # MSE tiles of [128, 4*512]
F32 = mybir.dt.float32
AX = mybir.AxisListType
AF = mybir.ActivationFunctionType
OP = mybir.AluOpType

_CACHE = {}


def _build(niters=N_ITERS, with_mse=True, with_ar=True, with_final=True):
    nc = bacc.Bacc("TRN2", target_bir_lowering=False, debug=False, num_devices=NCORES)

    x_ext = nc.dram_tensor("x", [NLOC, D_DATA], F32, kind="ExternalInput")
    dec_ext = nc.dram_tensor("dec", [NLOC, D_DATA], F32, kind="ExternalInput")
    enc_ext = nc.dram_tensor("enc", [NLOC, D_LAT], F32, kind="ExternalInput")
    enct_ext = nc.dram_tensor("enct", [D_LAT + 1, NLOC], F32, kind="ExternalInput")
    c0_ext = nc.dram_tensor("c0", [K, D_LAT], F32, kind="ExternalInput")
    stats_ext = nc.dram_tensor("stats", [128, 2], F32, kind="ExternalOutput")

    with tile.TileContext(nc) as tc:
        with (
            tc.tile_pool(name="singles", bufs=1) as singles,
            tc.tile_pool(name="small", bufs=2) as small,
            tc.tile_pool(name="stat8", bufs=3) as stat8,
            tc.tile_pool(name="ework", bufs=3) as ework,
            tc.tile_pool(name="scratch", bufs=2) as scratch,
            tc.tile_pool(name="msein", bufs=3) as msein,
            tc.tile_pool(name="psum_m", bufs=3, space="PSUM") as psum_m_pool,
            tc.tile_pool(name="psum_nd", bufs=2, space="PSUM") as psum_nd_pool,
            tc.tile_pool(name="psum_t", bufs=2, space="PSUM") as psum_t_pool,
            tc.tile_pool(name="dram", bufs=2, space="DRAM") as dram,
        ):
            # ---- resident setup ----
            enc_aug = singles.tile([128, NCHUNK, D_LAT + 1], F32)   # row chunks + ones col
            enct_aug = singles.tile([D_LAT + 1, NLOC], F32)         # enc.T with ones row
            c_cur = singles.tile([K, D_LAT], F32)
            identity = singles.tile([K, K], F32)
            x2col = singles.tile([128, NCHUNK], F32)                # |enc_n|^2 per sample
            z_final = singles.tile([128, NCHUNK], F32)
            s_tile = singles.tile([128, NCHUNK], F32)
            mse_cols = singles.tile([128, NMSE], F32)
            stats = singles.tile([128, 2], F32)

            enc_src = enc_ext[:].rearrange("(c p) d -> p c d", p=128)
            nc.sync.dma_start(out=enc_aug[:, :, 0:D_LAT], in_=enc_src)
            nc.vector.memset(enc_aug[:, :, D_LAT : D_LAT + 1], 1.0)
            nc.sync.dma_start(out=enct_aug, in_=enct_ext[:])
            nc.sync.dma_start(out=c_cur, in_=c0_ext[:])
            make_identity(nc, identity)

            x_src = x_ext[:].rearrange("(c p) d -> p c d", p=128)
            d_src = dec_ext[:].rearrange("(c p) d -> p c d", p=128)

            def emit_mse_tile(i):
                xt = msein.tile([128, 4, D_DATA], F32, tag="xt")
                dt = msein.tile([128, 4, D_DATA], F32, tag="dt")
                nc.sync.dma_start(out=xt, in_=x_src[:, 4 * i : 4 * i + 4, :])
                nc.sync.dma_start(out=dt, in_=d_src[:, 4 * i : 4 * i + 4, :])
                y = msein.tile([128, 4, D_DATA], F32, tag="y")
                nc.vector.tensor_tensor(out=y, in0=xt, in1=dt, op=OP.subtract)
                nc.scalar.activation(
                    out=y, in_=y, func=AF.Square,
                    accum_out=mse_cols[:, i : i + 1],
                )

            def emit_x2(c):
                sq = scratch.tile([128, D_LAT], F32, tag="x2sq")
                nc.scalar.activation(
                    out=sq, in_=enc_aug[:, c, 0:D_LAT], func=AF.Square,
                    accum_out=x2col[:, c : c + 1],
                )

            mse_emitted = 0

            # ---- k-means iterations ----
            for t in range(niters):
                last = with_final and (t == niters - 1)

                # Build Cmat [D+1, K]: rows 0..63 = 2*C.T, row 64 = -|C|^2
                caug = small.tile([K, D_LAT + 1], F32, tag="caug")
                c2t = small.tile([K, 1], F32, tag="c2t")
                csq = small.tile([K, D_LAT], F32, tag="csq")
                nc.vector.tensor_scalar_mul(caug[:, 0:D_LAT], c_cur, 2.0)
                nc.scalar.activation(
                    out=csq, in_=c_cur, func=AF.Square, accum_out=c2t
                )
                nc.vector.tensor_scalar_mul(caug[:, D_LAT : D_LAT + 1], c2t, -1.0)
                p_t = psum_t_pool.tile([D_LAT + 1, K], F32, tag="p_t")
                nc.tensor.transpose(p_t, caug, identity)
                cmat = small.tile([D_LAT + 1, K], F32, tag="cmat")
                nc.scalar.copy(cmat, p_t)

                if not last:
                    p_nd = psum_nd_pool.tile([K, D_LAT + 1], F32, tag="p_nd")

                for g in range(NCHUNK // NGRP):
                    p_m = psum_m_pool.tile([128, NGRP, K], F32, tag="p_m")
                    for j in range(NGRP):
                        c = g * NGRP + j
                        nc.tensor.matmul(
                            p_m[:, j, :],
                            lhsT=enct_aug[:, c * 128 : (c + 1) * 128],
                            rhs=cmat,
                            start=True, stop=True,
                        )
                    negmx = stat8.tile([128, NGRP], F32, tag="negmx")
                    nc.vector.reduce_max(negmx, p_m, axis=AX.X, negate=True)

                    et = ework.tile([128, NGRP, K], F32, tag="et")
                    if not last:
                        z8 = stat8.tile([128, NGRP], F32, tag="z8")
                        for j in range(NGRP):
                            nc.scalar.activation(
                                out=et[:, j, :], in_=p_m[:, j, :], func=AF.Exp,
                                bias=negmx[:, j : j + 1],
                                accum_out=z8[:, j : j + 1],
                            )
                        rec8 = stat8.tile([128, NGRP], F32, tag="rec8")
                        nc.vector.reciprocal(rec8, z8)
                        rt = ework.tile([128, NGRP, K], F32, tag="rt")
                        for j in range(NGRP):
                            nc.vector.tensor_scalar_mul(
                                rt[:, j, :], et[:, j, :], rec8[:, j : j + 1]
                            )
                        for j in range(NGRP):
                            c = g * NGRP + j
                            nc.tensor.matmul(
                                p_nd,
                                lhsT=rt[:, j, :],
                                rhs=enc_aug[:, c, :],
                                start=(c == 0), stop=(c == NCHUNK - 1),
                            )
                    else:
                        # loss-only pass: Z into z_final, s_c = sum_k e * min(m - x2, 0)
                        for j in range(NGRP):
                            c = g * NGRP + j
                            nc.scalar.activation(
                                out=et[:, j, :], in_=p_m[:, j, :], func=AF.Exp,
                                bias=negmx[:, j : j + 1],
                                accum_out=z_final[:, c : c + 1],
                            )
                        for j in range(NGRP):
                            c = g * NGRP + j
                            nd2 = scratch.tile([128, K], F32, tag="nd2")
                            nc.vector.tensor_scalar(
                                out=nd2, in0=p_m[:, j, :],
                                scalar1=x2col[:, c : c + 1], scalar2=0.0,
                                op0=OP.subtract, op1=OP.min,
                            )
                            pr = scratch.tile([128, K], F32, tag="pr")
                            nc.vector.tensor_tensor(
                                out=pr, in0=et[:, j, :], in1=nd2, op=OP.mult
                            )
                            nc.vector.tensor_reduce(
                                out=s_tile[:, c : c + 1], in_=pr,
                                axis=AX.X, op=OP.add,
                            )

                if not last:
                    # AllReduce the [K, D+1] numerator/denominator and update C
                    nd_sb = small.tile([K, D_LAT + 1], F32, tag="nd_sb")
                    nc.scalar.copy(nd_sb, p_nd)
                    nd_ar = small.tile([K, D_LAT + 1], F32, tag="nd_ar")
                    if with_ar:
                        cc_in = dram.tile([K, D_LAT + 1], F32, tag="cc_in")
                        cc_out = dram.tile([K, D_LAT + 1], F32, tag="cc_out")
                        nc.gpsimd.dma_start(out=cc_in, in_=nd_sb)
                        nc.gpsimd.collective_compute(
                            "AllReduce",
                            OP.add,
                            replica_groups=[list(range(NCORES))],
                            ins=[cc_in.opt()],
                            outs=[cc_out.opt()],
                        )
                        nc.gpsimd.dma_start(out=nd_ar, in_=cc_out)
                    else:
                        nc.vector.tensor_copy(out=nd_ar, in_=nd_sb)
                    dent = small.tile([K, 1], F32, tag="dent")
                    nc.vector.tensor_scalar_add(
                        dent, nd_ar[:, D_LAT : D_LAT + 1], EPS
                    )
                    recd = small.tile([K, 1], F32, tag="recd")
                    nc.vector.reciprocal(recd, dent)
                    nc.vector.tensor_scalar_mul(c_cur, nd_ar[:, 0:D_LAT], recd)

                # interleave MSE work into the AllReduce gaps
                if with_mse and t >= 1:
                    for _ in range(2):
                        if mse_emitted < NMSE:
                            emit_mse_tile(mse_emitted)
                            mse_emitted += 1
                # emit x2 (needed only by the last pass) late in the schedule
                if with_final and t == niters - 2:
                    for c in range(NCHUNK):
                        emit_x2(c)

            if with_final and niters == 1:
                for c in range(NCHUNK):
                    emit_x2(c)
            while with_mse and mse_emitted < NMSE:
                emit_mse_tile(mse_emitted)
                mse_emitted += 1

            # ---- epilogue: fold into stats [128, 2] ----
            if with_mse:
                nc.vector.tensor_reduce(
                    out=stats[:, 0:1], in_=mse_cols, axis=AX.X, op=OP.add
                )
            else:
                nc.vector.memset(stats[:, 0:1], 0.0)
            if with_final:
                recz = singles.tile([128, NCHUNK], F32)
                nc.vector.reciprocal(recz, z_final)
                lsc = scratch.tile([128, NCHUNK], F32, tag="lsc")
                nc.vector.tensor_tensor(out=lsc, in0=s_tile, in1=recz, op=OP.mult)
                nc.vector.tensor_reduce(
                    out=stats[:, 1:2], in_=lsc, axis=AX.X, op=OP.add
                )
            else:
                nc.vector.memset(stats[:, 1:2], 0.0)
            nc.sync.dma_start(out=stats_ext[:], in_=stats)

    nc.compile()
    return nc


def _get_nc():
    if "nc" not in _CACHE:
        _CACHE["nc"] = _build()
    return _CACHE["nc"]


def _run(X, encoding, decoding, trace=False, **trace_kwargs):
    X = np.ascontiguousarray(np.asarray(X, dtype=np.float32))
    enc = np.ascontiguousarray(np.asarray(encoding, dtype=np.float32))
    dec = np.ascontiguousarray(np.asarray(decoding, dtype=np.float32))
    assert X.shape == (N, D_DATA) and enc.shape == (N, D_LAT) and dec.shape == (N, D_DATA)

    nc = _get_nc()

    c0 = np.ascontiguousarray(enc[:K])  # replicated deterministic init
    in_maps = []
    for i in range(NCORES):
        sl = slice(i * NLOC, (i + 1) * NLOC)
        enct = np.empty((D_LAT + 1, NLOC), np.float32)
        enct[:D_LAT] = enc[sl].T
        enct[D_LAT] = 1.0
        in_maps.append(
            {
                "x": np.ascontiguousarray(X[sl]),
                "dec": np.ascontiguousarray(dec[sl]),
                "enc": np.ascontiguousarray(enc[sl]),
                "enct": enct,
                "c0": c0,
            }
        )

    res = run_bass_kernel_spmd(
        nc, in_maps, core_ids=list(range(NCORES)), trace=trace, **trace_kwargs
    )

    mse_sum = 0.0
    negloss_sum = 0.0
    for r in res.results:
        st = r["stats"].astype(np.float64)
        mse_sum += st[:, 0].sum()
        negloss_sum += st[:, 1].sum()
    value = mse_sum / (N * D_DATA) + ALPHA * (-negloss_sum / N)
    return np.float32(value), res


def kernel(X, encoding, decoding, K):
    assert int(K) == 64
    value, _ = _run(X, encoding, decoding, trace=False)
    return value
